# revision 14
# baseline (speedup 1.0000x reference)
"""Tensor-parallel attention kernel for Trainium2 (8 NeuronCores).

Problem: B=1, L=2048, D=4096, H=32 q-heads, KV=8 kv-heads, HD=128,
partial rotary ROT=64, causal additive mask, o-projection.

Sharding: TP-8 over heads. Core c owns q-heads 4c..4c+3 and kv-head c
(column shard of w_qkv), plus the matching row shard of w_o. Each core
computes a full [L, D] partial of the output; the host sums the 8
partials (the cross-core reduction of the row-sharded o-projection).

On-chip everything is bf16 (fp32 PSUM accumulation) in "transposed"
orientation so every matmul contracts over the partition dim:
  qkvT[col, L] = w_qkv.T @ x.T          (w stationary, xT streamed)
  rope:  qT' = qT * cosE + (P @ qT) * sinE   (P = rotate-half matrix on PE)
  ST[k, q]   = kT_tile.T @ qT            (one matmul per k-tile, K=HD=128)
  PT         = exp(ST + maskT)           (no max subtraction; exp(-1e9)=0)
  pacc      += PT                        (vector; k-tile partial sums)
  den[*, q]  = ones.T @ pacc             (one matmul per (jq, h))
  oT[d, q]   = V_tile.T @ PT             (V from a one-time PE transpose of vT)
  out[l, e]  = (oT/den).T @ w_o_shard    (partial; summed across cores on host)

Causal structure: 512-wide q blocks only touch k-tiles at/below the
diagonal; the 4 diagonal k-tiles use narrowed moving operands
(512/384/256/128 wide) plus a single constant 128x128 triangle mask.
The o-projection of q-block jq-1 is interleaved into q-block jq's
attention tile loop to keep the PE busy while exp runs on Scalar.
"""

import sys

for _p in ("/opt/trn_rl_repo", "/root/.axon_site/_ro/trn_rl_repo"):
    if _p not in sys.path:
        sys.path.append(_p)

import numpy as np

B, L, D = 1, 2048, 4096
H, KV, HD = 32, 8, 128
ROT = 64
SCALE = HD ** -0.5
NEG = -1e9
NCORES = 8
HPC = H // NCORES          # q-heads per core (4)
CPC = HPC * HD + 2 * HD    # w_qkv columns per core (768)
NDT = D // 128             # contraction tiles over D (32)
NKT = L // 128             # k tiles (16)
NJQ = L // 512             # 512-wide q blocks (4)
XB = 512                   # L-block width in the qkv phase
NLB = L // XB              # 4

_cache = {}


def _build(causal: bool):
    import concourse.mybir as mybir
    import concourse.tile as tile
    from concourse import bacc

    F32 = mybir.dt.float32
    F32R = mybir.dt.float32r
    BF16 = mybir.dt.bfloat16
    EXP = mybir.ActivationFunctionType.Exp

    nc = bacc.Bacc("TRN2", target_bir_lowering=False, debug=False)

    xt = nc.dram_tensor("xt", [D, L], BF16, kind="ExternalInput").ap()
    wqkv = nc.dram_tensor("wqkv", [D, CPC], BF16, kind="ExternalInput").ap()
    wo = nc.dram_tensor("wo", [HPC * HD, D], BF16, kind="ExternalInput").ap()
    cos_e = nc.dram_tensor("cos_e", [2, 128, L], F32, kind="ExternalInput").ap()
    sin_e = nc.dram_tensor("sin_e", [2, 128, L], F32, kind="ExternalInput").ap()
    # ident | pmat_t | tri  (bf16) and ones (f32)
    cbf = nc.dram_tensor("cbf", [128, 384], BF16, kind="ExternalInput").ap()
    cfr = nc.dram_tensor("cfr", [128, 128], F32, kind="ExternalInput").ap()
    if not causal:
        mask_t = nc.dram_tensor("mask_t", [L, L], BF16, kind="ExternalInput").ap()
    out_p = nc.dram_tensor("out_p", [L, D], F32, kind="ExternalOutput").ap()

    xt_r = xt.rearrange("(dt p) l -> p dt l", p=128)
    wqkv_r = wqkv.rearrange("(dt p) c -> p dt c", p=128)
    wo_r = wo.rearrange("(h p) e -> p h e", p=128)

    with tile.TileContext(nc) as tc:
        with tc.tile_pool(name="persist", bufs=1) as persist:
            kt_sb = persist.tile([128, L], BF16, tag="kt")
            v_sb = persist.tile([128, NKT, 128], BF16, tag="v")
            qt_sb = persist.tile([128, HPC, L], BF16, tag="qt")
            wo_sb = persist.tile([128, HPC, D], BF16, tag="wo")
            cbf_sb = persist.tile([128, 384], BF16, tag="cbf")
            ones_sb = persist.tile([128, 128], F32R, tag="ones")
            ident = cbf_sb[:, 0:128]
            pmat_t = cbf_sb[:, 128:256]
            tri = cbf_sb[:, 256:384]

            # ---------------- Phase 1: qkv projection + rope ----------------
            with tc.tile_pool(name="wq", bufs=1) as wqp, \
                 tc.tile_pool(name="xb", bufs=2) as xbp, \
                 tc.tile_pool(name="tabs", bufs=2) as tabs, \
                 tc.tile_pool(name="stage", bufs=3) as stage, \
                 tc.tile_pool(name="vstg", bufs=2) as vstg, \
                 tc.tile_pool(name="ps1", bufs=1, space="PSUM") as ps1:
                # weights/x in independent slab tiles so matmuls gate on
                # per-slab DMA completion, not the whole tensor; the first
                # slab is a single dti so the first matmul starts ASAP
                SIZES = [1, 3, 4, 4, 4, 4, 4, 4, 4]
                STARTS = [sum(SIZES[:k]) for k in range(len(SIZES))]
                SLAB_OF = []
                for sl, sz in enumerate(SIZES):
                    for k in range(sz):
                        SLAB_OF.append((sl, k))
                wqs = []
                for sl, sz in enumerate(SIZES):
                    wq_t = wqp.tile([128, sz, CPC], BF16, tag="wq%d" % sl)
                    wqs.append(wq_t)

                def xslabs():
                    out = []
                    for sl, sz in enumerate(SIZES):
                        x_t = xbp.tile([128, sz, XB], BF16, tag="xs%d" % sl)
                        out.append(x_t)
                    return out

                # lb=0: stripe x/wq slabs round-robin across the 3 DMA rings
                # (each ring is FIFO; first matmul gates on xs[0]+wqs[0])
                xs_cur = xslabs()
                ls0 = slice(0, XB)
                rings = [nc.sync, nc.scalar, nc.gpsimd]
                for sl, sz in enumerate(SIZES):
                    ds = slice(STARTS[sl], STARTS[sl] + sz)
                    rings[(2 * sl) % 3].dma_start(out=xs_cur[sl], in_=xt_r[:, ds, ls0])
                    rings[(2 * sl + 1) % 3].dma_start(out=wqs[sl], in_=wqkv_r[:, ds, :])
                nc.sync.dma_start(out=cbf_sb, in_=cbf)
                nc.sync.dma_start(out=ones_sb, in_=cfr.bitcast(F32R))
                cos_cur = tabs.tile([128, 2, XB], F32, tag="cosb")
                sin_cur = tabs.tile([128, 2, XB], F32, tag="sinb")
                nc.scalar.dma_start(out=cos_cur, in_=cos_e[:, :, ls0].rearrange("t p l -> p t l"))
                nc.scalar.dma_start(out=sin_cur, in_=sin_e[:, :, ls0].rearrange("t p l -> p t l"))

                for lb in range(NLB):
                    xs, cosb, sinb = xs_cur, cos_cur, sin_cur
                    if lb + 1 < NLB:
                        # prefetch next L-block (one compute block of slack)
                        ln = slice((lb + 1) * XB, (lb + 2) * XB)
                        xs_cur = xslabs()
                        for sl, sz in enumerate(SIZES):
                            ds = slice(STARTS[sl], STARTS[sl] + sz)
                            nc.gpsimd.dma_start(out=xs_cur[sl], in_=xt_r[:, ds, ln])
                        cos_cur = tabs.tile([128, 2, XB], F32, tag="cosb")
                        sin_cur = tabs.tile([128, 2, XB], F32, tag="sinb")
                        nc.scalar.dma_start(out=cos_cur, in_=cos_e[:, :, ln].rearrange("t p l -> p t l"))
                        nc.scalar.dma_start(out=sin_cur, in_=sin_e[:, :, ln].rearrange("t p l -> p t l"))
                    ls = slice(lb * XB, (lb + 1) * XB)
                    # dti-major over the 5 rope columns: each weight/x slab is
                    # consumed right as it lands, 5 open PSUM accumulations
                    accs = []
                    for ct in range(5):
                        acc_t = ps1.tile([128, XB], F32, tag="acc%d" % ct)
                        accs.append(acc_t)
                    for dti in range(NDT):
                        sl, so = SLAB_OF[dti]
                        for ct in range(5):
                            nc.tensor.matmul(
                                out=accs[ct],
                                lhsT=wqs[sl][:, so, ct * 128:(ct + 1) * 128],
                                rhs=xs[sl][:, so, :],
                                start=(dti == 0), stop=(dti == NDT - 1))
                    # v column (ct=5): its matmuls are issued as PE filler
                    # between the rope ops below
                    acc5 = ps1.tile([128, XB], F32, tag="acc5")
                    vi = [0]

                    def vfill(k):
                        while vi[0] < min(k, NDT):
                            d = vi[0]
                            dsl, dso = SLAB_OF[d]
                            nc.tensor.matmul(
                                out=acc5, lhsT=wqs[dsl][:, dso, 640:768],
                                rhs=xs[dsl][:, dso, :],
                                start=(d == 0), stop=(d == NDT - 1))
                            vi[0] += 1

                    # rope for q (ct 0..3, scaled tables) and k (ct 4)
                    for ct in range(5):
                        ti = 0 if ct < 4 else 1
                        s_sb = stage.tile([128, XB], BF16, tag="s_sb")
                        nc.scalar.copy(out=s_sb, in_=accs[ct])
                        vfill(6 * ct + 3)
                        rot = ps1.tile([128, XB], F32, tag="rot")
                        nc.tensor.matmul(out=rot, lhsT=pmat_t, rhs=s_sb,
                                         start=True, stop=True)
                        vfill(6 * ct + 6)
                        dst = kt_sb[:, ls] if ct == 4 else qt_sb[:, ct, ls]
                        nc.vector.tensor_mul(dst, s_sb, cosb[:, ti, :])
                        m2 = stage.tile([128, XB], BF16, tag="m2")
                        nc.vector.tensor_mul(m2, rot, sinb[:, ti, :])
                        nc.vector.tensor_add(dst, dst, m2)
                    vfill(NDT)
                    # v: copy to vT staging, transpose the block's four
                    # k-tiles into resident V
                    vt = vstg.tile([128, XB], BF16, tag="vt")
                    nc.scalar.copy(out=vt, in_=acc5)
                    for kk in range(4):
                        i = 4 * lb + kk
                        tp = ps1.tile([128, 128], BF16, tag="vtp")
                        nc.tensor.transpose(
                            tp, vt[:, kk * 128:(kk + 1) * 128], ident)
                        nc.vector.tensor_copy(v_sb[:, i, :], tp)
                    if lb == 0:
                        # prefetch w_o during the rest of phase 1
                        nc.scalar.dma_start(out=wo_sb, in_=wo_r)

            # ---------------- Phase 2+3: attention + o-projection ----------
            cm = [tc.tile_pool(name="pt", bufs=4),
                  tc.tile_pool(name="pacc", bufs=2),
                  tc.tile_pool(name="rd", bufs=2),
                  tc.tile_pool(name="otn", bufs=2),
                  tc.tile_pool(name="ost", bufs=6),
                  tc.tile_pool(name="ps_st", bufs=4, space="PSUM"),
                  tc.tile_pool(name="ps_ot", bufs=2, space="PSUM"),
                  tc.tile_pool(name="ps3", bufs=2, space="PSUM")]
            if not causal:
                cm.append(tc.tile_pool(name="mb", bufs=2))
            pools = [c.__enter__() for c in cm]
            ptp, paccp, rdp, otnp, ostp, ps_st, ps_ot, ps3 = pools[:8]
            mbp = pools[8] if not causal else None

            ost_flip = [0]
            ost_cur = [None]  # half-filled [128, 1024] write-combine tile

            def oproj_acc(jq_src, otn_src, lt, et):
                # et pairs (2e, 2e+1) share one [128,1024] tile -> one DMA
                acc3 = ps3.tile([128, XB], F32, tag="acc3")
                for hh in range(HPC):
                    nc.tensor.matmul(
                        out=acc3,
                        lhsT=otn_src[:, hh, lt * 128:(lt + 1) * 128],
                        rhs=wo_sb[:, hh, et * 512:(et + 1) * 512],
                        start=(hh == 0), stop=(hh == HPC - 1))
                if et % 2 == 0:
                    ost_t = ostp.tile([128, 2 * XB], F32, tag="ost")
                    ost_cur[0] = ost_t
                ost = ost_cur[0]
                half = ost[:, (et % 2) * XB:(et % 2 + 1) * XB]
                if ost_flip[0] % 2 == 0:
                    nc.vector.tensor_copy(half, acc3)
                else:
                    nc.scalar.copy(out=half, in_=acc3)
                ost_flip[0] += 1
                if et % 2 == 1:
                    r0 = jq_src * 512 + lt * 128
                    weng = nc.sync if (lt % 2 == 0) else nc.gpsimd
                    weng.dma_start(
                        out=out_p[r0:r0 + 128, (et - 1) * 512:(et + 1) * 512],
                        in_=ost)

            prev = None  # (jq index, otn tile)
            for jq in range(NJQ):
                qs0 = jq * 512
                otn_t = otnp.tile([128, HPC, 512], BF16, tag="otn")
                if not causal:
                    mblk = mbp.tile([128, NKT, 512], BF16, tag="mblk")
                    nc.sync.dma_start(
                        out=mblk,
                        in_=mask_t[:, qs0:qs0 + 512].rearrange("(kt p) q -> p kt q", p=128))
                pend = ([] if prev is None else
                        [(prev[0], prev[1], lt, et)
                         for lt in range(4) for et in range(8)])
                if causal:
                    per_h = [(i, 0, 512) for i in range(4 * jq)] + \
                            [(4 * jq + di, 128 * di, 512 - 128 * di)
                             for di in range(4)]
                else:
                    per_h = [(i, 0, 512) for i in range(NKT)]
                ntiles = HPC * len(per_h)
                tcount = 0
                npend0 = max(1, len(pend))

                for h in range(HPC):
                    pacc = paccp.tile([128, XB], F32R, tag="pacc")
                    ot = ps_ot.tile([128, XB], F32, tag="ot")
                    T = len(per_h)
                    pipe = []  # 2-deep: (qoff, i, pt tile)
                    ot_started = [False]

                    def ot_mm(last=False):
                        po, pi, ppt = pipe.pop(0)
                        nc.tensor.matmul(
                            out=ot[:, po:], lhsT=v_sb[:, pi, :],
                            rhs=ppt[:, po:], start=(not ot_started[0]),
                            stop=last)
                        ot_started[0] = True

                    for u, (i, qoff, w) in enumerate(per_h):
                        st = ps_st.tile([128, XB], F32, tag="st")
                        nc.tensor.matmul(
                            out=st[:, qoff:], lhsT=kt_sb[:, i * 128:(i + 1) * 128],
                            rhs=qt_sb[:, h, qs0 + qoff:qs0 + 512],
                            start=True, stop=True)
                        if len(pipe) >= 2:
                            ot_mm()
                        if causal:
                            if i >= 4 * jq:
                                nc.vector.tensor_add(
                                    st[:, qoff:qoff + 128],
                                    st[:, qoff:qoff + 128], tri)
                        else:
                            nc.vector.tensor_add(st, st, mblk[:, i, :])
                        pt_t = ptp.tile([128, XB], BF16, tag="pt")
                        nc.scalar.activation(pt_t[:, qoff:], st[:, qoff:], EXP)
                        if u == 0:
                            nc.vector.tensor_copy(pacc, pt_t)
                        else:
                            nc.vector.tensor_add(pacc[:, qoff:], pacc[:, qoff:],
                                                 pt_t[:, qoff:])
                        pipe.append((qoff, i, pt_t))
                        # interleave previous q-block's o-projection
                        tcount += 1
                        want = tcount * npend0 // ntiles
                        while pend and (npend0 - len(pend)) < want:
                            js, osrc, lt, et = pend.pop(0)
                            oproj_acc(js, osrc, lt, et)
                    while pipe:
                        ot_mm(last=(len(pipe) == 1))
                    # denominator rides the st PSUM ring (no extra bank)
                    dn = ps_st.tile([128, XB], F32, tag="st")
                    nc.tensor.matmul(out=dn, lhsT=ones_sb, rhs=pacc,
                                     start=True, stop=True)
                    rd = rdp.tile([128, XB], F32, tag="rd")
                    nc.vector.reciprocal_approx_fast(out=rd, in_=dn)
                    nc.vector.tensor_mul(otn_t[:, h, :], ot, rd)
                while pend:
                    js, osrc, lt, et = pend.pop(0)
                    oproj_acc(js, osrc, lt, et)
                prev = (jq, otn_t)

            # final o-projection for the last q block
            for lt in range(4):
                for et in range(8):
                    oproj_acc(prev[0], prev[1], lt, et)

            for c in reversed(cm):
                c.__exit__(None, None, None)

    nc.compile()
    return nc


def _host_inputs(x, attention_mask, cos, sin, w_qkv, w_o, causal):
    """Build the 8 per-core input maps."""
    import ml_dtypes
    bf16 = ml_dtypes.bfloat16

    xt = np.ascontiguousarray(x[0].T).astype(bf16)        # [D, L]
    q_pos = H * HD
    kv_pos = q_pos + KV * HD

    # extended rope tables [2, 128, L]: slot 0 = q (scale folded), slot 1 = k
    # row d<64: cos[l, d]; row d>=64: 1.0 (cos) / 0.0 (sin)
    cos_t = cos.T.astype(np.float32)                      # [ROT, L]
    sin_t = sin.T.astype(np.float32)
    cos_e = np.empty((2, 128, L), np.float32)
    sin_e = np.zeros((2, 128, L), np.float32)
    cos_e[0, :ROT] = cos_t * SCALE
    cos_e[0, ROT:] = SCALE
    cos_e[1, :ROT] = cos_t
    cos_e[1, ROT:] = 1.0
    sin_e[0, :ROT] = sin_t * SCALE
    sin_e[1, :ROT] = sin_t

    # pmat_t[d, d'] = Pmat[d', d]; rot[d'] = -x[d'+32] (d'<32), x[d'-32] (32<=d'<64)
    pmat = np.zeros((128, 128), np.float32)
    for dp in range(32):
        pmat[dp, dp + 32] = -1.0
    for dp in range(32, 64):
        pmat[dp, dp - 32] = 1.0
    # tri[kk, qj] = 0 if kk <= qj else NEG  (within-tile causal triangle)
    tri = np.where(np.triu(np.ones((128, 128), dtype=bool)),
                   np.float32(0.0), np.float32(NEG))
    cbf = np.concatenate(
        [np.eye(128, dtype=np.float32), pmat.T, tri], axis=1).astype(bf16)
    cfr = np.ones((128, 128), np.float32)

    mask2d = np.ascontiguousarray(attention_mask[0, 0])   # [L(q), L(k)]
    if causal:
        mask_t_full = None
    else:
        mask_t_full = np.ascontiguousarray(mask2d.T).astype(bf16)  # [k, q]

    in_maps = []
    for c in range(NCORES):
        cols = []
        for j in range(HPC):
            h = c * HPC + j
            cols.append(w_qkv[:, h * HD:(h + 1) * HD])
        cols.append(w_qkv[:, q_pos + c * HD:q_pos + (c + 1) * HD])
        cols.append(w_qkv[:, kv_pos + c * HD:kv_pos + (c + 1) * HD])
        wqkv_c = np.ascontiguousarray(
            np.concatenate(cols, axis=1)).astype(bf16)               # [D, 768]
        wo_c = np.ascontiguousarray(
            w_o[c * HPC * HD:(c + 1) * HPC * HD, :]).astype(bf16)    # [512, D]
        m = {"xt": xt, "wqkv": wqkv_c, "wo": wo_c,
             "cos_e": cos_e, "sin_e": sin_e, "cbf": cbf, "cfr": cfr}
        if not causal:
            m["mask_t"] = mask_t_full
        in_maps.append(m)
    return in_maps


def _is_causal(mask2d):
    expected = np.where(
        np.tril(np.ones((L, L), dtype=bool)), np.float32(0.0), np.float32(NEG))
    return mask2d.shape == (L, L) and np.array_equal(mask2d, expected)


def kernel(x, attention_mask, cos, sin, w_qkv, w_o, _trace=False):
    from concourse.bass_utils import run_bass_kernel_spmd

    x = np.asarray(x, dtype=np.float32)
    attention_mask = np.asarray(attention_mask, dtype=np.float32)
    cos = np.asarray(cos, dtype=np.float32)
    sin = np.asarray(sin, dtype=np.float32)
    w_qkv = np.asarray(w_qkv, dtype=np.float32)
    w_o = np.asarray(w_o, dtype=np.float32)

    causal = _is_causal(attention_mask[0, 0])
    if causal not in _cache:
        _cache[causal] = _build(causal)
    nc = _cache[causal]

    in_maps = _host_inputs(x, attention_mask, cos, sin, w_qkv, w_o, causal)
    try:
        res = run_bass_kernel_spmd(nc, in_maps, list(range(NCORES)), trace=_trace)
    except Exception:
        # transient device errors (e.g. NRT_EXEC_UNIT_UNRECOVERABLE) usually
        # clear on retry
        res = run_bass_kernel_spmd(nc, in_maps, list(range(NCORES)), trace=_trace)
    out = np.zeros((L, D), np.float64)
    for c in range(NCORES):
        out += res.results[c]["out_p"].astype(np.float64)
    if _trace:
        kernel._last_exec_time_ns = res.exec_time_ns
    return out.astype(np.float32).reshape(B, L, D)
